# revision 6
# baseline (speedup 1.0000x reference)
"""CalderaLinear fused kernel for 8 Trainium2 NeuronCores.

Math (reference): y = x @ Q^T + (x @ R^T) @ L^T + bias, with Q/L/R groupwise
int-dequantized (codes 0..15, group size 128).

Strategy:
  * Column-parallel over d_out: core c owns out-features [c*512, (c+1)*512).
  * On each core, first build W_c = Q_c^T + R^T @ L_c^T  ([d_in, 512]) on-chip:
    R and L^T are dequantized with DVE ops (codes arrive as exact bf16),
    R^T L^T accumulates on the PE into PSUM, and dequantized Q^T is added on
    eviction. Low-rank contraction (rank 256) makes this ~64 small matmuls.
  * Then y_c = x @ W_c + bias_c: x streams through as 128x512 pre-tiled
    blocks (host-side retiling gives one contiguous DMA per tile), W_c stays
    SBUF-resident, PSUM accumulates over the 32 k-tiles, bias is fused into
    the PSUM eviction.
  * Host side only reshapes/transposes/casts and concatenates the 8 output
    shards: all dequant + matmul math runs on the NeuronCores.

Compute dtype is bf16 (codes 0..15 are exact; verified rel-err ~2e-3 vs fp32
reference, dominated by bf16 rounding of x). Set CALDERA_DTYPE=float32r for
the reduced-precision-fp32 PE mode (~1.4e-4 rel-err, ~15% slower).
"""

import os
import numpy as np
import ml_dtypes

P = 128
D_IN = 4096
D_OUT = 4096
TOK = 8192
RANK = 256
NCORES = 8
OC = D_OUT // NCORES      # 512 out features per core
KT = D_IN // P            # 32 contraction tiles
MS = 512                  # token slab
NS = TOK // MS            # 16 slabs
SUB = MS // P             # 4 psum sub-tiles per slab
KG = D_IN // 128          # 32 scale groups along d_in
RG = RANK // 128          # 2 scale groups along rank

_module_cache = {}
last_result = None


def _build_module(dt_name):
    import concourse.mybir as mybir
    import concourse.tile as tile
    from concourse import bacc

    use_f32r = dt_name == "float32r"
    dt_c = mybir.dt.float32 if use_f32r else getattr(mybir.dt, dt_name)
    f32 = mybir.dt.float32

    def mm(ap):
        # PE consumes fp32 data in the fast reduced-precision float32r mode
        return ap.bitcast(mybir.dt.float32r) if use_f32r else ap

    nc = bacc.Bacc(None, target_bir_lowering=False, debug=False)
    xt_d = nc.dram_tensor("xt", (NS, KT, P, MS), dt_c, kind="ExternalInput")
    qt_d = nc.dram_tensor("qt", (D_IN, OC), dt_c, kind="ExternalInput")
    qst_d = nc.dram_tensor("qst", (1, KT, OC), f32, kind="ExternalInput")
    rv_d = nc.dram_tensor("rv", (RANK, D_IN), dt_c, kind="ExternalInput")
    rs_d = nc.dram_tensor("rs", (RANK, KG), f32, kind="ExternalInput")
    ltv_d = nc.dram_tensor("ltv", (RANK, OC), dt_c, kind="ExternalInput")
    lst_d = nc.dram_tensor("lst", (1, RG, OC), f32, kind="ExternalInput")
    bias_d = nc.dram_tensor("biasv", (1, OC), f32, kind="ExternalInput")
    y_d = nc.dram_tensor("y", (TOK, OC), f32, kind="ExternalOutput")

    with tile.TileContext(nc) as tc:
        with (
            tc.tile_pool(name="const", bufs=1) as const,
            tc.tile_pool(name="wpool", bufs=1) as wpool,
            tc.tile_pool(name="xpool", bufs=8) as xpool,
            tc.tile_pool(name="qpool", bufs=3) as qpool,
            tc.tile_pool(name="bpool", bufs=3) as bpool,
            tc.tile_pool(name="ypool", bufs=8) as ypool,
            tc.tile_pool(name="ppool", bufs=8, space="PSUM") as ppool,
        ):
            # ---- small constant loads
            rvt = const.tile([P, RG, D_IN], dt_c)
            rst = const.tile([P, RG, KG], f32)
            ltvt = const.tile([P, RG, OC], dt_c)
            lstrow = const.tile([1, RG, OC], f32)
            qstrow = const.tile([1, KT, OC], f32)
            biasrow = const.tile([1, OC], f32)
            bias_t = const.tile([P, OC], f32)
            for j in range(RG):
                nc.sync.dma_start(rvt[:, j, :], rv_d[j * P:(j + 1) * P, :])
                nc.sync.dma_start(rst[:, j, :], rs_d[j * P:(j + 1) * P, :])
                nc.sync.dma_start(ltvt[:, j, :], ltv_d[j * P:(j + 1) * P, :])
            nc.sync.dma_start(lstrow[:], lst_d[:])
            nc.sync.dma_start(qstrow[:], qst_d[:])
            nc.sync.dma_start(biasrow[:], bias_d[:])
            nc.gpsimd.partition_broadcast(bias_t[:], biasrow[:])

            # ---- dequantize R ([rank, d_in], scale per (r, i-group))
            Rd = const.tile([P, RG, D_IN], dt_c)
            for j in range(RG):
                for g in range(KG):
                    nc.vector.tensor_scalar_mul(
                        Rd[:, j, g * 128:(g + 1) * 128],
                        rvt[:, j, g * 128:(g + 1) * 128],
                        rst[:, j, g:g + 1],
                    )

            # ---- dequantize L^T ([rank, oc], scale per (o, r-group))
            LdT = const.tile([P, RG, OC], dt_c)
            for j in range(RG):
                lb = bpool.tile([P, OC], f32, tag="bc", name=f"lb{j}")
                nc.gpsimd.partition_broadcast(lb[:], lstrow[0:1, j, :])
                nc.vector.tensor_mul(LdT[:, j, :], ltvt[:, j, :], lb[:])

            # ---- W_c = R^T @ L^T + Q^T  (k-tile at a time)
            Wt = wpool.tile([P, KT, OC], dt_c)
            for k in range(KT):
                ps = ppool.tile([P, OC], f32, tag="ps", name=f"wb{k}")
                for j in range(RG):
                    nc.tensor.matmul(
                        ps[:],
                        mm(Rd[:, j, k * P:(k + 1) * P]),
                        mm(LdT[:, j, :]),
                        start=(j == 0),
                        stop=(j == RG - 1),
                    )
                qc = qpool.tile([P, OC], dt_c, tag="qc")
                nc.sync.dma_start(qc[:], qt_d[k * P:(k + 1) * P, :])
                qb = bpool.tile([P, OC], f32, tag="bc", name=f"qb{k}")
                nc.gpsimd.partition_broadcast(qb[:], qstrow[0:1, k, :])
                qdq = qpool.tile([P, OC], dt_c, tag="qd")
                nc.vector.tensor_mul(qdq[:], qc[:], qb[:])
                nc.vector.tensor_add(Wt[:, k, :], ps[:], qdq[:])

            # ---- main matmul: y = x @ W + bias
            for s in range(NS):
                psums = [
                    ppool.tile([P, OC], f32, tag="ps", name=f"ps{s}_{i}")
                    for i in range(SUB)
                ]
                for k in range(KT):
                    xt = xpool.tile([P, MS], dt_c, tag="x")
                    nc.sync.dma_start(xt[:], xt_d[s, k])
                    for sub in range(SUB):
                        nc.tensor.matmul(
                            psums[sub][:],
                            mm(xt[:, sub * P:(sub + 1) * P]),
                            mm(Wt[:, k, :]),
                            start=(k == 0),
                            stop=(k == KT - 1),
                        )
                for sub in range(SUB):
                    yt = ypool.tile([P, OC], f32, tag="y")
                    nc.vector.tensor_add(yt[:], psums[sub][:], bias_t[:])
                    nc.sync.dma_start(
                        y_d[s * MS + sub * P:s * MS + (sub + 1) * P, :], yt[:]
                    )

    nc.compile()
    return nc


def kernel(x, q_values, q_scales, l_values, l_scales, r_values, r_scales, bias,
           _trace=False):
    from concourse.bass_utils import run_bass_kernel_spmd

    dt_name = os.environ.get("CALDERA_DTYPE", "bfloat16")
    np_in = ml_dtypes.bfloat16 if dt_name == "bfloat16" else np.float32

    if dt_name not in _module_cache:
        _module_cache[dt_name] = _build_module(dt_name)
    nc = _module_cache[dt_name]

    # host-side marshaling (shapes only; all math runs on-device)
    x = np.asarray(x, dtype=np.float32)
    # xt[s, k, p, m] = x[s*MS + m, k*P + p]
    xt = np.ascontiguousarray(
        x.reshape(NS, MS, KT, P).transpose(0, 2, 3, 1)
    ).astype(np_in)
    rv = np.ascontiguousarray(r_values).astype(np_in)
    rs = np.ascontiguousarray(r_scales).astype(np.float32)

    in_maps = []
    for c in range(NCORES):
        sl = slice(c * OC, (c + 1) * OC)
        in_maps.append({
            "xt": xt,
            "qt": np.ascontiguousarray(q_values[sl].T).astype(np_in),
            "qst": np.ascontiguousarray(q_scales[sl].T).reshape(1, KT, OC).astype(np.float32),
            "rv": rv,
            "rs": rs,
            "ltv": np.ascontiguousarray(l_values[sl].T).astype(np_in),
            "lst": np.ascontiguousarray(l_scales[sl].T).reshape(1, RG, OC).astype(np.float32),
            "biasv": np.ascontiguousarray(bias[sl]).reshape(1, OC).astype(np.float32),
        })

    res = run_bass_kernel_spmd(
        nc, in_maps, core_ids=list(range(NCORES)), trace=_trace
    )
    global last_result
    last_result = res
    return np.concatenate([r["y"] for r in res.results], axis=1)


# revision 13
# speedup vs baseline: 1.0267x; 1.0267x over previous
"""CalderaLinear fused kernel for 8 Trainium2 NeuronCores.

Math (reference): y = x @ Q^T + (x @ R^T) @ L^T + bias, with Q/L/R groupwise
int-dequantized (codes 0..15, group size 128).

Strategy:
  * Column-parallel over d_out: core c owns out-features [c*512, (c+1)*512).
  * On each core, first build W_c = Q_c^T + R^T @ L_c^T  ([d_in, 512]) on-chip:
    R and L^T are dequantized with DVE ops (codes arrive as exact bf16),
    R^T L^T accumulates on the PE into PSUM, and dequantized Q^T is added on
    eviction. Low-rank contraction (rank 256) makes this ~64 small matmuls.
  * Then y_c = x @ W_c + bias_c: x streams through as 128x512 pre-tiled
    blocks (host-side retiling gives one contiguous DMA per tile), W_c stays
    SBUF-resident, PSUM accumulates over the 32 k-tiles, bias is fused into
    the PSUM eviction.
  * Host side only reshapes/transposes/casts and concatenates the 8 output
    shards: all dequant + matmul math runs on the NeuronCores.

Compute dtype is bf16 (codes 0..15 are exact; verified rel-err ~2e-3 vs fp32
reference, dominated by bf16 rounding of x). Set CALDERA_DTYPE=float32r for
the reduced-precision-fp32 PE mode (~1.4e-4 rel-err, ~15% slower).
"""

import os
import numpy as np
import ml_dtypes

P = 128
D_IN = 4096
D_OUT = 4096
TOK = 8192
RANK = 256
NCORES = 8
OC = D_OUT // NCORES      # 512 out features per core
KT = D_IN // P            # 32 contraction tiles
MS = 512                  # token slab
NS = TOK // MS            # 16 slabs
SUB = MS // P             # 4 psum sub-tiles per slab
KG = D_IN // 128          # 32 scale groups along d_in
RG = RANK // 128          # 2 scale groups along rank

_module_cache = {}
last_result = None


def _build_module(dt_name):
    import concourse.mybir as mybir
    import concourse.tile as tile
    from concourse import bacc

    use_f32r = dt_name == "float32r"
    dt_c = mybir.dt.float32 if use_f32r else getattr(mybir.dt, dt_name)
    f32 = mybir.dt.float32

    def mm(ap):
        # PE consumes fp32 data in the fast reduced-precision float32r mode
        return ap.bitcast(mybir.dt.float32r) if use_f32r else ap

    nc = bacc.Bacc(None, target_bir_lowering=False, debug=False)
    xt_d = nc.dram_tensor("xt", (NS, KT, P, MS), dt_c, kind="ExternalInput")
    qt_d = nc.dram_tensor("qt", (D_IN, OC), dt_c, kind="ExternalInput")
    # scales arrive pre-broadcast across partitions (host-side layout only)
    qst_d = nc.dram_tensor("qst", (KT, P, OC), dt_c, kind="ExternalInput")
    rv_d = nc.dram_tensor("rv", (RANK, D_IN), dt_c, kind="ExternalInput")
    rs_d = nc.dram_tensor("rs", (RANK, KG), f32, kind="ExternalInput")
    ltv_d = nc.dram_tensor("ltv", (RANK, OC), dt_c, kind="ExternalInput")
    lst_d = nc.dram_tensor("lst", (RG, P, OC), dt_c, kind="ExternalInput")
    bias_d = nc.dram_tensor("biasv", (P, OC), f32, kind="ExternalInput")
    y_d = nc.dram_tensor("y", (TOK, OC), f32, kind="ExternalOutput")

    with tile.TileContext(nc) as tc:
        with (
            tc.tile_pool(name="const", bufs=1) as const,
            tc.tile_pool(name="wpool", bufs=1) as wpool,
            tc.tile_pool(name="xpool", bufs=8) as xpool,
            tc.tile_pool(name="qpool", bufs=3) as qpool,
            tc.tile_pool(name="bpool", bufs=3) as bpool,
            tc.tile_pool(name="ypool", bufs=8) as ypool,
            tc.tile_pool(name="ppool", bufs=8, space="PSUM") as ppool,
        ):
            # ---- small constant loads
            rvt = const.tile([P, RG, D_IN], dt_c)
            rst = const.tile([P, RG, KG], f32)
            ltvt = const.tile([P, RG, OC], dt_c)
            bias_t = const.tile([P, OC], f32)
            for j in range(RG):
                nc.sync.dma_start(rvt[:, j, :], rv_d[j * P:(j + 1) * P, :])
                nc.sync.dma_start(rst[:, j, :], rs_d[j * P:(j + 1) * P, :])
                nc.sync.dma_start(ltvt[:, j, :], ltv_d[j * P:(j + 1) * P, :])
            nc.sync.dma_start(bias_t[:], bias_d[:])

            # ---- dequantize R ([rank, d_in], scale per (r, i-group))
            Rd = const.tile([P, RG, D_IN], dt_c)
            for j in range(RG):
                for g in range(KG):
                    nc.vector.tensor_scalar_mul(
                        Rd[:, j, g * 128:(g + 1) * 128],
                        rvt[:, j, g * 128:(g + 1) * 128],
                        rst[:, j, g:g + 1],
                    )

            # ---- dequantize L^T ([rank, oc], scale per (o, r-group))
            LdT = const.tile([P, RG, OC], dt_c)
            for j in range(RG):
                lb = bpool.tile([P, OC], dt_c, tag="bc", name=f"lb{j}")
                nc.sync.dma_start(lb[:], lst_d[j])
                nc.vector.tensor_mul(LdT[:, j, :], ltvt[:, j, :], lb[:])

            # ---- W_c = R^T @ L^T + Q^T  (k-tile at a time)
            Wt = wpool.tile([P, KT, OC], dt_c)
            for k in range(KT):
                ps = ppool.tile([P, OC], f32, tag="ps", name=f"wb{k}")
                for j in range(RG):
                    nc.tensor.matmul(
                        ps[:],
                        mm(Rd[:, j, k * P:(k + 1) * P]),
                        mm(LdT[:, j, :]),
                        start=(j == 0),
                        stop=(j == RG - 1),
                    )
                qc = qpool.tile([P, OC], dt_c, tag="qc")
                nc.sync.dma_start(qc[:], qt_d[k * P:(k + 1) * P, :])
                qb = bpool.tile([P, OC], dt_c, tag="bc", name=f"qb{k}")
                nc.sync.dma_start(qb[:], qst_d[k])
                qdq = qpool.tile([P, OC], dt_c, tag="qd")
                nc.vector.tensor_mul(qdq[:], qc[:], qb[:])
                nc.vector.tensor_add(Wt[:, k, :], ps[:], qdq[:])

            # ---- main matmul: y = x @ W + bias
            for s in range(NS):
                psums = [
                    ppool.tile([P, OC], f32, tag="ps", name=f"ps{s}_{i}")
                    for i in range(SUB)
                ]
                for k in range(KT):
                    xt = xpool.tile([P, MS], dt_c, tag="x")
                    nc.sync.dma_start(xt[:], xt_d[s, k])
                    for sub in range(SUB):
                        nc.tensor.matmul(
                            psums[sub][:],
                            mm(xt[:, sub * P:(sub + 1) * P]),
                            mm(Wt[:, k, :]),
                            start=(k == 0),
                            stop=(k == KT - 1),
                        )
                for sub in range(SUB):
                    yt = ypool.tile([P, OC], f32, tag="y")
                    nc.vector.tensor_add(yt[:], psums[sub][:], bias_t[:])
                    nc.sync.dma_start(
                        y_d[s * MS + sub * P:s * MS + (sub + 1) * P, :], yt[:]
                    )

    nc.compile()
    return nc


def kernel(x, q_values, q_scales, l_values, l_scales, r_values, r_scales, bias,
           _trace=False):
    from concourse.bass_utils import run_bass_kernel_spmd

    dt_name = os.environ.get("CALDERA_DTYPE", "bfloat16")
    np_in = ml_dtypes.bfloat16 if dt_name == "bfloat16" else np.float32

    if dt_name not in _module_cache:
        _module_cache[dt_name] = _build_module(dt_name)
    nc = _module_cache[dt_name]

    # host-side marshaling (shapes only; all math runs on-device)
    x = np.asarray(x, dtype=np.float32)
    # xt[s, k, p, m] = x[s*MS + m, k*P + p]
    xt = np.ascontiguousarray(
        x.reshape(NS, MS, KT, P).transpose(0, 2, 3, 1)
    ).astype(np_in)
    rv = np.ascontiguousarray(r_values).astype(np_in)
    rs = np.ascontiguousarray(r_scales).astype(np.float32)

    in_maps = []
    for c in range(NCORES):
        sl = slice(c * OC, (c + 1) * OC)
        in_maps.append({
            "xt": xt,
            "qt": np.ascontiguousarray(q_values[sl].T).astype(np_in),
            "qst": np.ascontiguousarray(
                np.broadcast_to(q_scales[sl].T.reshape(KT, 1, OC), (KT, P, OC))
            ).astype(np_in),
            "rv": rv,
            "rs": rs,
            "ltv": np.ascontiguousarray(l_values[sl].T).astype(np_in),
            "lst": np.ascontiguousarray(
                np.broadcast_to(l_scales[sl].T.reshape(RG, 1, OC), (RG, P, OC))
            ).astype(np_in),
            "biasv": np.ascontiguousarray(
                np.broadcast_to(bias[sl].reshape(1, OC), (P, OC))
            ).astype(np.float32),
        })

    res = run_bass_kernel_spmd(
        nc, in_maps, core_ids=list(range(NCORES)), trace=_trace
    )
    global last_result
    last_result = res
    return np.concatenate([r["y"] for r in res.results], axis=1)
